# revision 17
# baseline (speedup 1.0000x reference)
"""Multi-head causal attention (B=4, T=2048, C=1024, H=16) on 8 trn2 cores.

Sharding: core = (batch b, head-half hg): each core computes QKV for batch b
and its 8 heads, causal attention (scores kept transposed [key, query] so
softmax denominators come from an appended ones-column in V), and a partial
output projection over its 512 y-features. Host sums the two partial
projections per batch and adds b_proj.

Schedule: the scalar engine (exp, ~158us) and tensor engine (~225us) are the
binding resources. P1 (QKV), P3 (proj) and the softmax-denominator normalize
chain are interleaved into P2's per-score-block loop as filler units with
deadlines so the tensor engine runs continuously and never blocks the scalar
exp stream. Diagonal score blocks are trimmed to the causal triangle.
Weights/x are loaded as single consolidated DMAs to minimize trigger cost.
"""

import numpy as np
import ml_dtypes
import concourse.bass as bass
import concourse.mybir as mybir
import concourse.tile as tile
from concourse import bacc
from concourse.bass_utils import run_bass_kernel_spmd

B, T, C = 4, 2048, 1024
H, D = 16, 64
F32 = mybir.dt.float32
BF16 = mybir.dt.bfloat16
AFT = mybir.ActivationFunctionType

_CACHE = {}


def build():
    nc = bacc.Bacc(None, target_bir_lowering=False)
    xt_d = nc.dram_tensor("xt", [4, 128, 4096], BF16, kind="ExternalInput")
    wq_d = nc.dram_tensor("wq", [128, 4096], BF16, kind="ExternalInput")
    wk_d = nc.dram_tensor("wk", [128, 4096], BF16, kind="ExternalInput")
    wv_d = nc.dram_tensor("wv", [128, 4096], BF16, kind="ExternalInput")
    bqk_d = nc.dram_tensor("bqk", [128, 8], F32, kind="ExternalInput")
    tri2_d = nc.dram_tensor("tri2", [128, 256], BF16, kind="ExternalInput")
    wp_d = nc.dram_tensor("wp", [128, 4096], BF16, kind="ExternalInput")
    out_d = nc.dram_tensor("out", [T, C], BF16, kind="ExternalOutput")

    with nc.allow_low_precision(reason="bf16 matmul pipeline"):
        with tile.TileContext(nc) as tc:
            with (
                tc.tile_pool(name="const", bufs=1) as constp,
                tc.tile_pool(name="w1", bufs=1) as w1p,
                tc.tile_pool(name="x", bufs=1) as xp,
                tc.tile_pool(name="qk", bufs=1) as qkp,
                tc.tile_pool(name="vpool", bufs=1) as vp,
                tc.tile_pool(name="esb", bufs=1) as ep,
                tc.tile_pool(name="small", bufs=1) as smallp,
                tc.tile_pool(name="sps", bufs=1, space="PSUM") as spsp,
                tc.tile_pool(name="yps", bufs=1, space="PSUM") as ypsp,
                tc.tile_pool(name="aux", bufs=2, space="PSUM") as auxp,
            ):
                # ---- startup loads: first-needed pieces split over all 3
                # DMA-capable queues (wq/wk are packed ft-major by the host) ----
                bqk_t = constp.tile([128, 8], F32, tag="bqk")
                nc.sync.dma_start(bqk_t[:], bqk_d[:])
                tri2_t = constp.tile([128, 256], BF16, tag="tri2")
                nc.sync.dma_start(tri2_t[:], tri2_d[:])

                # ---- startup loads: first-needed pieces split over all 3
                # DMA-capable queues (wq/wk are packed ft-major by the host) ----
                wq_t = [w1p.tile([128, 1024], BF16, tag=f"wq{f}", name=f"wq{f}")
                        for f in range(4)]
                wk_t = [w1p.tile([128, 1024], BF16, tag=f"wk{f}", name=f"wk{f}")
                        for f in range(4)]
                wv_t = w1p.tile([128, 4096], BF16, tag="wv")
                wp_t = w1p.tile([128, 4096], BF16, tag="wp")
                x0 = xp.tile([128, 4096], BF16, tag="xt", bufs=2, name="x0")
                nc.sync.dma_start(x0[:, 0:1536], xt_d[0, :, 0:1536])
                nc.scalar.dma_start(x0[:, 1536:3072], xt_d[0, :, 1536:3072])
                nc.gpsimd.dma_start(x0[:, 3072:4096], xt_d[0, :, 3072:4096])
                for ft in range(4):
                    nc.sync.dma_start(wq_t[ft][:],
                                      wq_d[:, ft * 1024:(ft + 1) * 1024])
                    nc.scalar.dma_start(wk_t[ft][:],
                                        wk_d[:, ft * 1024:(ft + 1) * 1024])
                nc.gpsimd.dma_start(wv_t[:, 0:2048], wv_d[:, 0:2048])
                nc.scalar.dma_start(wv_t[:, 2048:4096], wv_d[:, 2048:4096])

                # persistent tensors
                qT = [qkp.tile([128, T], BF16, tag=f"qT{j}", name=f"qT{j}") for j in range(4)]
                kT = [qkp.tile([128, T], BF16, tag=f"kT{j}", name=f"kT{j}") for j in range(4)]
                yT = [qkp.tile([128, T], BF16, tag=f"yT{j}", name=f"yT{j}") for j in range(4)]
                vS = [vp.tile([128, 520], BF16, tag=f"v{t}", name=f"v{t}") for t in range(16)]
                # ones column of V (written once; P1 writes only cols 0:64 per head)
                for t in range(16):
                    vv = vS[t][:].rearrange("p (h c) -> p h c", c=65)
                    nc.gpsimd.memset(vv[:, :, 64:65], 1.0)

                # ---------- filler-unit machinery ----------
                # each unit is atomic (opens and closes its own psum group)
                def p1_chunk_units(nt):
                    """P1 for tokens [512*nt, 512*nt+512). Returns list of
                    (deadline, fn); deadline=(qt,pj,sc) = P2 iteration before
                    which the unit must be emitted; None = paced only."""
                    units = []
                    ts0 = nt * 512
                    xt_t = []

                    def dma_x():
                        if nt == 0:
                            xt_t.append(x0)
                            return
                        xx = xp.tile([128, 4096], BF16, tag="xt", bufs=2, name="xx")
                        nc.gpsimd.dma_start(xx[:], xt_d[nt])
                        xt_t.append(xx)
                    units.append(((nt, 0, 0, 0), dma_x))

                    def qk_unit(which, ft):
                        wt = wq_t[ft] if which == 0 else wk_t[ft]
                        dst = qT[ft] if which == 0 else kT[ft]
                        bcol = ft if which == 0 else 4 + ft

                        def fn():
                            xx = xt_t[0]
                            ps = auxp.tile([128, 512], F32, tag="aux", name="auxps")
                            for c in range(8):
                                nc.tensor.matmul(
                                    ps[:],
                                    wt[:, c * 128:(c + 1) * 128],
                                    xx[:, c * 512:(c + 1) * 512],
                                    start=(c == 0), stop=(c == 7))
                            nc.vector.tensor_scalar_add(
                                dst[:, ts0:ts0 + 512], ps[:], bqk_t[:, bcol:bcol + 1])
                        return fn

                    def v_unit(t2):
                        def fn():
                            xx = xt_t[0]
                            ps = auxp.tile([128, 512], F32, tag="aux", name="auxps")
                            for c in range(8):
                                nc.tensor.matmul(
                                    ps[:],
                                    xx[:, c * 512 + t2 * 128:c * 512 + t2 * 128 + 128],
                                    wv_t[:, c * 512:(c + 1) * 512],
                                    start=(c == 0), stop=(c == 7))
                            vv = vS[nt * 4 + t2][:].rearrange("p (h c) -> p h c", c=65)
                            nc.vector.tensor_copy(
                                vv[:, :, 0:64],
                                ps[:].rearrange("p (h c) -> p h c", c=64))
                        return fn

                    for ft in range(4):
                        units.append(((nt, ft, 0, 0), qk_unit(0, ft)))
                        units.append(((nt, ft, 0, 0), qk_unit(1, ft)))
                    for t2 in range(4):
                        units.append(((nt, 0, t2, 1), v_unit(t2)))
                    return units

                def norm_units(qt, pj, ysbs, coll, rec2):
                    """Normalize head pair pj of query tile qt (divide by the
                    softmax denominators collected in coll)."""
                    q0 = qt * 512
                    units = []

                    rrow_box = []

                    def recip():
                        nc.vector.reciprocal_approx_fast(rec2[:], coll[:])
                        rrow = smallp.tile([1, 512], F32, tag="rrow", bufs=4,
                                           name="rrow")
                        nc.sync.dma_start(rrow[:], rec2[1:2, :])
                        rrow_box.append(rrow)
                    units.append((None, recip))

                    def mk(h):
                        def fn():
                            srow = rec2[0:1, :] if h == 0 else rrow_box[0][:]
                            rb = smallp.tile([64, 512], F32, tag="rb", bufs=4,
                                             name="rb")
                            nc.gpsimd.partition_broadcast(rb[:], srow)
                            nc.vector.tensor_mul(
                                yT[pj][64 * h:64 * h + 64, q0:q0 + 512],
                                ysbs[2 * pj + h][0:64, :], rb[:])
                        return fn
                    for h in range(2):
                        units.append((None, mk(h)))
                    return units

                def p3_units(qt):
                    """Projection for token blocks of query tile qt (needs yT
                    cols [512qt, 512qt+512) normalized)."""
                    units = []

                    def mm_unit(tt, of):
                        def fn():
                            ps = auxp.tile([128, 512], F32, tag="aux", name="auxps")
                            for cy in range(4):
                                nc.tensor.matmul(
                                    ps[:], yT[cy][:, tt * 128:(tt + 1) * 128],
                                    wp_t[:, cy * 1024 + of * 512:cy * 1024 + of * 512 + 512],
                                    start=(cy == 0), stop=(cy == 3))
                            o_t = smallp.tile([128, 512], BF16, tag="osb", bufs=3,
                                              name="osb")
                            nc.vector.tensor_copy(o_t[:], ps[:])
                            nc.sync.dma_start(
                                out_d[tt * 128:(tt + 1) * 128,
                                      of * 512:(of + 1) * 512], o_t[:])
                        return fn

                    for tt in range(4 * qt, 4 * qt + 4):
                        for of in range(2):
                            units.append((None, mm_unit(tt, of)))
                    return units

                def p3a_units(qt, parts):
                    """First 3/4 of the qt projection (heads of pj 0-2), kept
                    in SBUF so only the pj3 contribution remains at the tail."""
                    units = []

                    def mm_unit(tt, of):
                        def fn():
                            ps = auxp.tile([128, 512], F32, tag="aux", name="auxps")
                            for cy in range(3):
                                nc.tensor.matmul(
                                    ps[:], yT[cy][:, tt * 128:(tt + 1) * 128],
                                    wp_t[:, cy * 1024 + of * 512:cy * 1024 + of * 512 + 512],
                                    start=(cy == 0), stop=(cy == 2))
                            o_p = smallp.tile([128, 512], F32, tag=f"opart{(tt % 4) * 2 + of}",
                                              bufs=1, name="opart")
                            nc.vector.tensor_copy(o_p[:], ps[:])
                            parts[(tt, of)] = o_p
                        return fn

                    for tt in range(4 * qt, 4 * qt + 4):
                        for of in range(2):
                            units.append((None, mm_unit(tt, of)))
                    return units

                def p3b_units(qt, parts):
                    units = []

                    def mm_unit(tt, of):
                        def fn():
                            ps = auxp.tile([128, 512], F32, tag="aux", name="auxps")
                            nc.tensor.matmul(
                                ps[:], yT[3][:, tt * 128:(tt + 1) * 128],
                                wp_t[:, 3 * 1024 + of * 512:3 * 1024 + of * 512 + 512],
                                start=True, stop=True)
                            o_t = smallp.tile([128, 512], BF16, tag="osb", bufs=3,
                                              name="osb")
                            nc.vector.tensor_add(o_t[:], ps[:], parts[(tt, of)][:])
                            nc.sync.dma_start(
                                out_d[tt * 128:(tt + 1) * 128,
                                      of * 512:(of + 1) * 512], o_t[:])
                        return fn

                    for tt in range(4 * qt, 4 * qt + 4):
                        for of in range(2):
                            units.append((None, mm_unit(tt, of)))
                    return units

                # ---------- emission ----------
                def dma_wp():
                    nc.gpsimd.dma_start(wp_t[:], wp_d[:])

                pending = list(p1_chunk_units(0))
                pending.insert(1, (None, dma_wp))

                def emit_due(cur):
                    i = 0
                    while i < len(pending):
                        dl, fn = pending[i]
                        if dl is not None and dl <= cur:
                            fn()
                            pending.pop(i)
                        else:
                            i += 1

                def emit_paced(n):
                    for _ in range(min(n, len(pending))):
                        pending.pop(0)[1]()

                p3parts = {}
                for qt in range(4):
                    q0 = qt * 512
                    ext = 4 * (qt + 1)
                    if qt < 3:
                        pending.extend(p1_chunk_units(qt + 1))

                    ysbs = {}
                    for pj in range(4):
                        coll = smallp.tile([2, 512], F32, tag=f"coll{pj}{qt % 2}",
                                           bufs=1, name="coll")
                        rec2 = smallp.tile([2, 512], F32, tag=f"rec2{pj}{qt % 2}",
                                           bufs=1, name="rec2")
                        y_ps = [ypsp.tile([65, 512], F32, tag=f"yps{h}", bufs=1,
                                          name=f"yps{h}")
                                for h in range(2)]
                        for sc in range(ext):
                            emit_due((qt, pj, sc, 0))
                            r = sc - (ext - 4)
                            off = 0 if r < 0 else 128 * r
                            s_ps = spsp.tile([128, 1024], F32, tag="sps", bufs=2,
                                             name="sps")
                            nc.tensor.matmul(s_ps[:, off:512],
                                             kT[pj][0:64, sc * 128:(sc + 1) * 128],
                                             qT[pj][0:64, q0 + off:q0 + 512],
                                             start=True, stop=True, tile_position=(0, 0))
                            nc.tensor.matmul(s_ps[:, 512 + off:1024],
                                             kT[pj][64:128, sc * 128:(sc + 1) * 128],
                                             qT[pj][64:128, q0 + off:q0 + 512],
                                             start=True, stop=True, tile_position=(64, 0))
                            e_t = ep.tile([128, 1024], BF16, tag="e", bufs=4, name="e")
                            e3 = e_t[:].rearrange("p (h q) -> p h q", h=2)
                            s3 = s_ps[:].rearrange("p (h q) -> p h q", h=2)
                            nc.scalar.activation(e3[:, :, off:512], s3[:, :, off:512],
                                                 AFT.Exp, scale=0.125)
                            emit_due((qt, pj, sc, 1))
                            if r >= 0:
                                tr3 = tri2_t[:].rearrange("p (h q) -> p h q", h=2)
                                nc.vector.tensor_mul(e3[:, :, off:off + 128],
                                                     e3[:, :, off:off + 128], tr3[:, :, :])
                            for h in range(2):
                                hc = 130 * pj + 65 * h
                                nc.tensor.matmul(y_ps[h][:, off:512],
                                                 vS[sc][:, hc:hc + 65],
                                                 e_t[:, 512 * h + off:512 * h + 512],
                                                 start=(sc == 0), stop=(sc == ext - 1))
                            emit_paced(1)
                        # drain y to SBUF, stash denominators
                        for h in range(2):
                            i = 2 * pj + h
                            y_sb = smallp.tile([65, 512], F32, tag=f"ysb{i}", bufs=1,
                                               name=f"ysb{i}")
                            nc.vector.tensor_copy(y_sb[:], y_ps[h][:])
                            nc.sync.dma_start(coll[h:h + 1, :], y_sb[64:65, :])
                            ysbs[i] = y_sb
                        pending.extend(norm_units(qt, pj, ysbs, coll, rec2))
                        if qt == 3 and pj == 2:
                            pending.extend(p3a_units(3, p3parts))
                    # projection of this qt interleaves into qt+1
                    if qt < 3:
                        pending.extend(p3_units(qt))
                    else:
                        pending.extend(p3b_units(3, p3parts))
                # tail
                while pending:
                    pending.pop(0)[1]()

    if not nc.is_finalized():
        nc.finalize()
    return nc


def _get_nc():
    if "nc" not in _CACHE:
        _CACHE["nc"] = build()
    return _CACHE["nc"]


def kernel(x, w_attn, b_attn, w_proj, b_proj, _trace=False, _trace_kwargs=None):
    x = np.asarray(x, dtype=np.float32)
    w_attn = np.asarray(w_attn, dtype=np.float32)
    b_attn = np.asarray(b_attn, dtype=np.float32)
    w_proj = np.asarray(w_proj, dtype=np.float32)
    b_proj = np.asarray(b_proj, dtype=np.float32)

    bf = ml_dtypes.bfloat16

    def pack_w(w):  # [1024, 512] -> [128, 4096] (c-block major columns)
        return np.ascontiguousarray(
            w.reshape(8, 128, 512).transpose(1, 0, 2).reshape(128, 4096)).astype(bf)

    def pack_w_ft(w):  # [1024, 512] -> [128, 4096] (ft-major: ft*1024 + c*128 + j)
        # w[c*128+p, ft*128+j] -> out[p, ft*1024 + c*128 + j]
        return np.ascontiguousarray(
            w.reshape(8, 128, 4, 128).transpose(1, 2, 0, 3).reshape(128, 4096)
        ).astype(bf)

    tri = (np.arange(128)[:, None] <= np.arange(128)[None, :]).astype(np.float32)
    tri2 = np.concatenate([tri, tri], axis=1).astype(bf)
    in_maps = []
    for core in range(8):
        b, hg = core // 2, core % 2
        cs = hg * 512
        bq = b_attn[cs:cs + 512]
        bk = b_attn[C + cs:C + cs + 512]
        bqk = np.concatenate([bq.reshape(4, 128).T, bk.reshape(4, 128).T],
                             axis=1).astype(np.float32)
        # xt: [4 chunks, 128, 8*512]: chunk nt, partition p=c_lo, col c*512+t
        xt4 = np.ascontiguousarray(
            x[b].T.reshape(8, 128, 4, 512).transpose(2, 1, 0, 3).reshape(4, 128, 4096)
        ).astype(bf)
        # wp: [512, 1024] -> [128, 4096] (cy-block major)
        wp4 = np.ascontiguousarray(
            w_proj[cs:cs + 512, :].reshape(4, 128, 1024).transpose(1, 0, 2)
            .reshape(128, 4096)).astype(bf)
        in_maps.append({
            "xt": xt4,
            "wq": pack_w_ft(w_attn[:, cs:cs + 512]),
            "wk": pack_w_ft(w_attn[:, C + cs:C + cs + 512]),
            "wv": pack_w(w_attn[:, 2 * C + cs:2 * C + cs + 512]),
            "bqk": bqk,
            "tri2": tri2,
            "wp": wp4,
        })

    kw = {}
    if _trace:
        kw["trace"] = True
        if _trace_kwargs:
            kw.update(_trace_kwargs)
    res = run_bass_kernel_spmd(_get_nc(), in_maps, list(range(8)), **kw)
    _CACHE["last_results"] = res
    outs = [res.results[c]["out"].astype(np.float32) for c in range(8)]
    y = np.stack([outs[2 * b] + outs[2 * b + 1] for b in range(B)])
    beff = (b_proj.astype(np.float64)
            + b_attn[2 * C:].astype(np.float64) @ w_proj.astype(np.float64))
    return (y + beff.reshape(1, 1, C).astype(np.float32)).astype(np.float32)
